# revision 15
# baseline (speedup 1.0000x reference)
"""Trainium2 Bass kernel: GNN conv block (nn_Conv_block_49331994362308).

Computes, for N=100000 nodes with K=16 neighbors each:
    nh  = ij[:, :, 0]                      # [N, K] neighbor ids
    xnj = mean(x[nh], axis=1)              # neighbor-feature mean  [N, 128]
    xej = mean(e, axis=1)                  # edge-feature mean      [N, 64]
    out = relu(x @ Wc.T + xnj @ Wn.T + xej @ We.T)

Distribution: data-parallel over nodes across 8 NeuronCores (12500 nodes
per core, padded to 12544 = 98*128). x is replicated to every core (in
bf16) so the random neighbor gather x[nh] is a core-local indirect DMA
from HBM.

v2 pipeline (all DMA-visible tensors bf16 except the f32 output; the
kernel is chip-HBM-bandwidth-bound, so bytes are the metric):
  - Neighbor rows arrive via InstDMAGatherAnt on bf16 x directly (no
    f32->bf16 cast stage). dma_gather indices are int16, so x is viewed
    as [N/4, 4, 128] super-rows; the host buckets each tile's 2048 edges
    by nh%4 into fixed 640-slot segments, and one gather instruction per
    (7-tile chunk, class) covers 4480 slots (SWDGE cost is
    994ns + 0.34ns/descriptor, so big gathers amortize the fixed part).
  - Mean-pool of the scrambled gathered rows is 20 accumulating PE
    matmuls per tile against a one-hot matrix P[slot, node] which is
    generated ON DEVICE by a single DVE is_equal over broadcast access
    patterns (iota[n] == sid[slot]); sid is a tiny [slot] -> node id
    stream from the host, stored doubled so every operand keeps a packed
    2-byte innermost axis (DVE 2x mode). Pool output is xnjT [f, n] in
    PSUM; ACT copies it to SBUF bf16.
  - The edge-feature mean never materializes: out.T accumulates
    8 matmuls of We-replicated [kf, fo] against e.T blocks [kf, n]
    (sum over k and f in one PSUM chain), plus Wc@x.T and Wn@xnjT.
    1/K is pre-folded into Wn/We on the host. Output is produced
    transposed [f_out, node]; the host untransposes.
  - DVE applies ReLU into a staging buffer flushed once per 14-tile
    chunk to a once-written per-chunk DRAM tensor (no WAW chains).

Walrus's TRN2 queue-DMA codegen only supports ONE sync-wait command per
DMA (and one per PE LDWEIGHTS), so the structure keeps every DMA at a
single dependency front: indices/sids are preloaded once into SBUF, the
8 SWDGE bookkeeping lanes are warmed with dummy transfers that absorb
the preload front, and warm matmuls give PE a single-wait view of the
weight constants.
"""

from contextlib import ExitStack

import numpy as np

import concourse.bass as bass
import concourse.mybir as mybir
import concourse.tile as tile
from concourse.bass_utils import run_bass_kernel_spmd
from concourse.masks import make_identity
from concourse import library_config

P = 128
K = 16
XN_IN = 128
XE_IN = 64
XN_OUT = 128
N_CORES = 8
N_FULL = 100000
N_LOC = N_FULL // N_CORES          # 12500
N_LOC_PAD = ((N_LOC + P - 1) // P) * P  # 12544
CHUNK = 14                          # tiles per output chunk (98 = 7*14)
CT = 7                              # tiles per gather chunk (98 = 14*7)

F32 = mybir.dt.float32
BF16 = mybir.dt.bfloat16
F8 = mybir.dt.float8e4   # e features; exactness not needed at 2e-2 tolerance
I32 = mybir.dt.int32
I16 = mybir.dt.int16

NCLS = 4           # x rows per int16 "super-row" (mod classes)
# Padded gather slots per (tile, class). 544 = mean 512 + headroom; the
# host rebalances nodes across tiles so no bucket exceeds it. 544 % 128
# = 32, so a tile's class segment always spans exactly NBLK=5 physical
# 128-slot blocks (with halves shared between adjacent tiles; the
# one-hot zeroes foreign slots), keeping PE/DVE work identical to an
# aligned grid while cutting gather descriptors and bytes by 15%.
SEG = 544
import math as _math
_G = _math.gcd(SEG, P)
# worst-case 128-slot blocks a tile's class segment touches
NBLK = max((r + SEG + P - 1) // P for r in range(0, P, _G))
CH_T = NBLK * NCLS  # pool blocks per tile = 20
EB = K * XE_IN // P  # e.T blocks per tile = 8


def _stream_len(ct: int) -> int:
    """Gather slots per (chunk, class), rounded to full 128-slot blocks so
    the last output block is fully written (trailing pads gather row 0)."""
    return -(-ct * SEG // P) * P


def _chunks(n_tiles: int, chunk: int = CHUNK) -> list[int]:
    out = []
    t = 0
    while t < n_tiles:
        out.append(min(chunk, n_tiles - t))
        t += chunk
    return out


def build_program(
    n_loc_pad: int, n_src: int, ct: int = CT, chunk: int = CHUNK, reps: int = 1
) -> bass.Bass:
    """Build the SPMD per-core Bass program (same program on every core).

    reps > 1 duplicates every tile body (identical I/O) — a timing probe:
    wall(reps=2) - wall(reps=1) isolates one pass's true execution time.
    """
    assert n_loc_pad % P == 0 and n_src % NCLS == 0
    n_tiles = n_loc_pad // P
    assert n_tiles % ct == 0
    chunks = _chunks(n_tiles, chunk)
    n_gch = n_tiles // ct
    sl = _stream_len(ct)
    seg16 = sl // 16  # idx16 columns per (gather chunk, class)

    # detect_race_conditions=False: the post-schedule wait-legalizer's nop
    # carriers share scratch tiles and trip the sim race detector's
    # bookkeeping (same-engine program order makes them safe).
    nc = bass.Bass("TRN2", debug=False, detect_race_conditions=False)

    x_full = nc.dram_tensor("x_full", [n_src, XN_IN], BF16, kind="ExternalInput").ap()
    x_selfT = nc.dram_tensor("x_selfT", [P, n_loc_pad], BF16, kind="ExternalInput").ap()
    # e.T blocks: e_loc[p, t*EB*P + blk*P + n] = e[t*P+n, k, f], kf = blk*P+p
    e_loc = nc.dram_tensor("e_loc", [P, n_tiles * EB * P], F8, kind="ExternalInput").ap()
    # int16 super-row ids (nh//4), wrapped [16, L/16] + replicated to 128
    # partitions, concatenated over (gather chunk, class)
    idx_loc = nc.dram_tensor(
        "idx_loc", [P, n_gch * NCLS * seg16], I16, kind="ExternalInput"
    ).ap()
    # per-slot node ids (-1 for dead slots), doubled along the free axis so
    # the one-hot is_equal keeps a packed 2-byte innermost pair on every
    # operand (DVE 2x mode): sid[p, t*2*CH_T + 2*b + r] = node of slot
    # (b*128+p) of tile t, r in {0,1}
    sid_loc = nc.dram_tensor(
        "sid_loc", [P, n_tiles * 2 * CH_T], BF16, kind="ExternalInput"
    ).ap()
    iota_in = nc.dram_tensor("iota_in", [P, P], BF16, kind="ExternalInput").ap()
    wcT = nc.dram_tensor("wcT", [XN_IN, XN_OUT], BF16, kind="ExternalInput").ap()
    wnT = nc.dram_tensor("wnT", [XN_IN, XN_OUT], BF16, kind="ExternalInput").ap()
    # We.T/K duplicated on partitions: we2[p, fo] = We[fo, p % 64] / K
    we2 = nc.dram_tensor("we2", [P, XN_OUT], BF16, kind="ExternalInput").ap()
    # per-chunk outputs, transposed: out_c[p, i*128+n] = out[(t0+i)*128+n, p]
    outs = [
        nc.dram_tensor(f"out{c}", [P, cti * P], BF16, kind="ExternalOutput").ap()
        for c, cti in enumerate(chunks)
    ]

    nop_sem = nc.alloc_semaphore("waitnop")

    with tile.TileContext(nc) as tc, ExitStack() as ctx:
        nc.gpsimd.sem_clear(range(nop_sem.num, nop_sem.num + 1))
        nc.gpsimd.load_library(library_config.mlp)
        consts = ctx.enter_context(tc.tile_pool(name="consts", bufs=1))
        ident_bf = consts.tile([P, P], BF16, tag="ident_bf")
        make_identity(nc, ident_bf[:])
        wcT_sb = consts.tile([XN_IN, XN_OUT], BF16, tag="wc")
        wnT_sb = consts.tile([XN_IN, XN_OUT], BF16, tag="wn")
        we2_sb = consts.tile([P, XN_OUT], BF16, tag="we2")
        iota_sb = consts.tile([P, P], BF16, tag="iota")
        nc.sync.dma_start(wcT_sb[:], wcT[:, :])
        nc.sync.dma_start(wnT_sb[:], wnT[:, :])
        nc.sync.dma_start(we2_sb[:], we2[:, :])
        nc.sync.dma_start(iota_sb[:], iota_in[:, :])
        idx_all = consts.tile([P, n_gch * NCLS * seg16], I16, tag="idx_all")
        nc.sync.dma_start(idx_all[:], idx_loc[:, :])
        sid_all = consts.tile([P, n_tiles * 2 * CH_T], BF16, tag="sid_all")
        nc.sync.dma_start(sid_all[:], sid_loc[:, :])
        # x viewed as [n_src/4, 4, 128]: class j gathers row 4*i16+j via
        # elem_step=512 elements (1024B stride) and a j*128-element offset
        x4 = x_full.rearrange("(r c) f -> r c f", c=NCLS)

        # Warm the 8 SWDGE bookkeeping lanes: each dummy absorbs the
        # idx-preload front so later gathers carry only their PE front.
        scratch = ctx.enter_context(tc.tile_pool(name="scratch", bufs=1))
        for q in range(8):
            sc = scratch.tile([1, K], I16, tag=f"sc{q}")
            nc.gpsimd.dma_start(sc[:], idx_all[:1, :K])
        # Tiny template instructions for _legalize_waits nop carriers
        # (one per DMA queue and per compute engine).
        nop_hw = scratch.tile([1, K], I16, tag="noptpl_hw")
        nc.sync.dma_start(nop_hw[:], idx_loc[:1, :K])
        nop_sw = scratch.tile([1, K], I16, tag="noptpl_sw")
        nc.gpsimd.dma_start(nop_sw[:], idx_loc[:1, :K])
        nop_sw_act = scratch.tile([1, K], I16, tag="noptpl_sw_act")
        nc.scalar.dma_start(nop_sw_act[:], idx_loc[:1, :K])
        nop_sw_dve = scratch.tile([1, K], I16, tag="noptpl_sw_dve")
        nc.vector.dma_start(nop_sw_dve[:], idx_loc[:1, :K])
        nop_dve = scratch.tile([P, K], BF16, tag="noptpl_dve")
        nc.vector.tensor_copy(nop_dve[:], ident_bf[:, :K])
        nop_act = scratch.tile([P, K], BF16, tag="noptpl_act")
        nc.scalar.copy(nop_act[:], ident_bf[:, :K])
        nop_pool = scratch.tile([P, K], F32, tag="noptpl_pool")
        nc.gpsimd.memset(nop_pool[:], 0.0)

        g_pool = ctx.enter_context(tc.tile_pool(name="gatherp", bufs=2))
        pp_pool = ctx.enter_context(tc.tile_pool(name="poolmat", bufs=3))
        e_pool = ctx.enter_context(tc.tile_pool(name="edgep", bufs=3))
        xs_pool = ctx.enter_context(tc.tile_pool(name="xselfp", bufs=4))
        st_pool = ctx.enter_context(tc.tile_pool(name="stagep", bufs=3))
        out_pool = ctx.enter_context(tc.tile_pool(name="outp", bufs=2))
        psum_pool = ctx.enter_context(tc.tile_pool(name="psump", bufs=2, space="PSUM"))
        psum1_pool = ctx.enter_context(tc.tile_pool(name="psum1p", bufs=1, space="PSUM"))

        # Warm up PE's view of the constants so steady-state matmuls carry at
        # most one sync wait (PE LDWEIGHTS supports a single wait command).
        ps_warm = psum1_pool.tile([P, P], F32, tag="warm")
        nc.tensor.matmul(ps_warm[:], ident_bf[:], ident_bf[:], start=True, stop=False)
        nc.tensor.matmul(ps_warm[:], wcT_sb[:], ident_bf[:], start=False, stop=False)
        nc.tensor.matmul(ps_warm[:], wnT_sb[:], ident_bf[:], start=False, stop=False)
        nc.tensor.matmul(ps_warm[:], we2_sb[:], ident_bf[:], start=False, stop=True)

        t = 0
        gb = [None] * NCLS
        nidx_reg = nc.gpsimd.to_reg(sl)  # shared across all gathers
        for c, cti in enumerate(chunks):
            o_stage = out_pool.tile([P, cti * P], BF16, tag="ostage")
            for i0 in range(cti * reps):
                rep, i = divmod(i0, cti) if reps > 1 else (0, i0)
                t = (c * chunk) + i
                g, ti = divmod(t, ct)

                if ti == 0:
                    # one gather per class covering this ct-tile chunk;
                    # slot s lands at partition s%128, free block s//128, so
                    # each tile's 5 blocks per class stay 128-aligned.
                    for j in range(NCLS):
                        off = (g * NCLS + j) * seg16
                        gout = g_pool.tile([P, sl], BF16, tag=f"go{j}")
                        nc.gpsimd.dma_gather(
                            out_ap=gout[:].rearrange("p (b f) -> p b f", f=XN_IN),
                            in_ap=x4[:, j, :],
                            idxs_ap=idx_all[:, off:off + seg16],
                            num_idxs=sl,
                            num_idxs_reg=nidx_reg,
                            elem_size=XN_IN,
                            elem_step=NCLS * XN_IN,
                            single_packet=False,
                        )
                        gb[j] = gout

                # one-hot P[slot, node] = (iota[node] == sid[slot]), one DVE
                # op per tile over broadcast APs with packed innermost pairs
                p_t = pp_pool.tile([P, CH_T * P], BF16, tag="pmat")
                o_ap = p_t[:].rearrange("p (b nh nl) -> p b nh nl", b=CH_T, nl=2)
                i_ap = (
                    iota_sb[:]
                    .rearrange("p (o nh nl) -> p o nh nl", o=1, nl=2)
                    .to_broadcast([P, CH_T, P // 2, 2])
                )
                s_ap = (
                    sid_all[:, t * 2 * CH_T:(t + 1) * 2 * CH_T]
                    .rearrange("p (b nl) -> p b nl", nl=2)
                    .unsqueeze(2)
                    .to_broadcast([P, CH_T, P // 2, 2])
                )
                nc.vector.tensor_tensor(
                    o_ap, i_ap, s_ap, op=mybir.AluOpType.is_equal
                )

                # xnjT[f, n] = sum_slot g[slot, f] * P[slot, n]
                xnjT_ps = psum_pool.tile([P, P], F32, tag="ps_xnj")
                for b in range(CH_T):
                    j, bl = divmod(b, NBLK)
                    blk = (SEG * ti) // P + bl
                    nc.tensor.matmul(
                        xnjT_ps[:],
                        gb[j][:, blk * XN_IN:(blk + 1) * XN_IN],
                        p_t[:, b * P:(b + 1) * P],
                        start=(b == 0),
                        stop=(b == CH_T - 1),
                    )
                xnjT_bf = st_pool.tile([P, P], BF16, tag="xnj_bf")
                nc.scalar.copy(xnjT_bf[:], xnjT_ps[:])

                x_sb = xs_pool.tile([P, P], BF16, tag="xT")
                nc.vector.dma_start(x_sb[:], x_selfT[:, t * P:(t + 1) * P])
                e_sb = e_pool.tile([P, EB * P], F8, tag="e")
                nc.scalar.dma_start(
                    e_sb[:], e_loc[:, t * EB * P:(t + 1) * EB * P]
                )

                # outT[fo, n] = sum_f wcT[f,fo] xT[f,n] + wnT[f,fo] xnjT[f,n]
                #             + sum_kf we2[kf,fo] eT[kf,n]
                outT_ps = psum_pool.tile([P, P], F32, tag="ps_out")
                for blk in range(EB):
                    nc.tensor.matmul(
                        outT_ps[:],
                        we2_sb[:],
                        e_sb[:, blk * P:(blk + 1) * P],
                        start=(blk == 0),
                        stop=False,
                    )
                nc.tensor.matmul(outT_ps[:], wcT_sb[:], x_sb[:], start=False, stop=False)
                nc.tensor.matmul(outT_ps[:], wnT_sb[:], xnjT_bf[:], start=False, stop=True)

                # ReLU on DVE into the chunk staging buffer
                nc.vector.tensor_scalar_max(
                    o_stage[:, i * P:(i + 1) * P], outT_ps[:], 0.0
                )

            nc.sync.dma_start(outs[c][:, :], o_stage[:])

    from concourse.library_overlay import lower_extended_insts

    lower_extended_insts(nc)
    _legalize_waits(nc, nop_sem)
    return nc


def _legalize_waits(nc: bass.Bass, nop_sem) -> None:
    """Split multi-wait queue-DMAs / matmuls for walrus's 1-wait codegen limit.

    The TRN2 walrus codegen allows a single sync-wait command per queue-DMA
    entry and per PE matmul (S3_LW struct). Tile emits minimal waits but can
    still produce 2+ (e.g. a slot's previous-writer DMA completion plus its
    last-reader engine release — Tile's clocks are not transitive). Queue
    entries execute in FIFO order, so extra waits are moved onto tiny no-op
    carrier DMAs inserted immediately before the offender on the same queue.
    For matmuls the carrier is a 1-column bf16 LDWEIGHTS (any clobbered
    weights are reloaded by each matmul's own weight load; insertion happens
    before a directly-preceding LDWEIGHTS so split LDW+MM pairs stay intact).
    """
    import copy

    dma_tpl: dict = {}
    eng_tpl: dict = {}
    evsem_tpl: dict = {}
    ldw_tpl = None
    for f in nc.m.functions:
        for blk in f.blocks:
            for inst in blk.instructions:
                tn = type(inst).__name__
                dst = (
                    str(getattr(inst.outs[0], "memref", "")) if inst.outs else ""
                )
                if tn == "InstDMACopy":
                    if dst.startswith("nop_hw"):
                        dma_tpl["qSPDynamicHW"] = inst
                    elif dst.startswith("nop_sw"):
                        dma_tpl[inst.queue] = inst
                elif tn == "InstLdweights" and ldw_tpl is None:
                    ldw_tpl = inst
                elif tn == "InstEventSemaphore":
                    evsem_tpl[inst.engine] = inst
                elif dst.startswith("nop_dve") or dst.startswith("nop_act") or dst.startswith("nop_pool"):
                    eng_tpl[inst.engine] = inst

    counter = [0]

    def make_nop(tpl, wait):
        counter[0] += 1
        nop = copy.deepcopy(tpl)
        nop.name = f"I-{nc.next_id()}"
        # DMA carriers must update a semaphore (BIR invariant); use a
        # dedicated one nobody waits on. Other engines' carriers stay
        # update-free (walrus rejects a waitnop update on e.g. TensorCopy
        # with a no_semaphore_value_conflict ISA check).
        upd = []
        if type(tpl).__name__ == "InstDMACopy":
            upd = [
                mybir.SyncUpdate(
                    sync_type="semaphore",
                    id=nop_sem.num,
                    ant_name=nop_sem.name,
                    update_mode="sem-add-imm",
                    update_value=16,
                )
            ]
        nop.sync_info = mybir.SyncInfo(on_wait=[wait], on_update=upd)
        nc.inst_map[nop.name] = nop
        return nop

    for f in nc.m.functions:
        for blk in f.blocks:
            out: list = []
            changed = False
            insts = list(blk.instructions)
            for pos, inst in enumerate(insts):
                tn = type(inst).__name__
                si = inst.sync_info
                waits = list(si.on_wait) if si else []
                nops = None
                if len(waits) > 1:
                    if tn == "InstDMACopy":
                        tpl = dma_tpl.get(inst.queue)
                        assert tpl is not None, f"no nop template for {inst.queue}"
                        nops = [make_nop(tpl, w) for w in waits[:-1]]
                    elif tn in ("InstMatmult", "InstLdweights"):
                        assert ldw_tpl is not None, "no ldweights template"
                        nops = [make_nop(ldw_tpl, w) for w in waits[:-1]]
                        # keep split LDW+MM pairs adjacent
                        if out and type(out[-1]).__name__ == "InstLdweights":
                            own_ldw = out.pop()
                            nops.append(own_ldw)
                    elif tn == "InstDrain":
                        # a drain is its own carrier: extra single-wait drains
                        # on the same engine are harmless
                        nops = [make_nop(inst, w) for w in waits[:-1]]
                    elif inst.engine in eng_tpl and tn not in (
                        "InstDrain",
                        "InstEventSemaphore",
                        "InstSemaphoreOp",
                    ):
                        nops = [make_nop(eng_tpl[inst.engine], w) for w in waits[:-1]]
                if nops:
                    out.extend(nops)
                    inst.sync_info = mybir.SyncInfo(
                        on_wait=waits[-1:], on_update=list(si.on_update)
                    )
                    changed = True
                out.append(inst)
            if changed:
                try:
                    blk.instructions[:] = out
                except TypeError:
                    blk.instructions.clear()
                    blk.instructions.extend(out)


_PROGRAM_CACHE: dict = {}


def _get_program(n_loc_pad: int, n_src: int, ct: int = CT, chunk: int = CHUNK) -> bass.Bass:
    key = (n_loc_pad, n_src, ct, chunk)
    if key not in _PROGRAM_CACHE:
        _PROGRAM_CACHE[key] = build_program(n_loc_pad, n_src, ct, chunk)
    return _PROGRAM_CACHE[key]


def prep_gather(nh_pad: np.ndarray, ct: int = CT):
    """Bucket edges by nh%4 per tile into fixed SEG-slot segments of the
    per-(chunk, class) gather stream; emit the int16 super-row stream
    (wrapped [16, L/16], replicated to 128 partitions) and the doubled
    per-slot node-id stream for on-device one-hot generation. Tile
    segments need not be 128-aligned: sid maps each of the NBLK touched
    physical blocks, with -1 for slots owned by the adjacent tile.

    Returns (idx16 [128, n_gch*NCLS*seg16], sid2 [128, n_tiles*2*CH_T] f32).
    """
    n_pad = nh_pad.shape[0]
    n_tiles = n_pad // P
    assert n_tiles % ct == 0
    n_gch = n_tiles // ct
    sl = _stream_len(ct)
    seg16 = sl // 16

    idx16 = np.zeros((n_gch * NCLS, sl), np.int16)
    sid = np.full((n_tiles, CH_T * P), -1.0, np.float32)  # [tile, slot] -> node
    for t in range(n_tiles):
        nh_t = nh_pad[t * P:(t + 1) * P]          # [128 nodes, K]
        nodes = np.repeat(np.arange(P), K)         # edge -> node
        vals = nh_t.reshape(-1)                    # edge -> neighbor id
        cls = vals % NCLS
        g, ti = divmod(t, ct)
        base = ti * SEG           # segment start within the chunk stream
        sb = (SEG * ti) // P      # first physical block the tile touches
        for j in range(NCLS):
            sel = np.nonzero(cls == j)[0]
            l = len(sel)
            assert l <= SEG, f"class overflow {l} > {SEG}"
            idx16[g * NCLS + j, base:base + l] = (vals[sel] // NCLS).astype(
                np.int16
            )
            o = np.arange(l)
            pb = (base + o) // P          # physical block of each slot
            pp = (base + o) % P           # partition of each slot
            sid[t, (j * NBLK + pb - sb) * P + pp] = nodes[sel]
    # wrap idx16: entry i -> [i%16, i//16]; replicate 16-row block to 128
    idx16 = idx16.reshape(n_gch * NCLS, sl // 16, 16).transpose(0, 2, 1)
    idx16 = np.tile(idx16, (1, 8, 1)).reshape(n_gch, NCLS, P, seg16)
    idx16 = np.ascontiguousarray(
        idx16.transpose(2, 0, 1, 3).reshape(P, n_gch * NCLS * seg16)
    )
    # sid2[p, t*2*CH_T + 2*b + r] = sid[t, b*128 + p]
    sid_b = sid.reshape(n_tiles, CH_T, P)          # [t, b, p]
    sid2 = np.repeat(sid_b, 2, axis=1).reshape(n_tiles, CH_T, 2, P)
    sid2 = np.ascontiguousarray(
        sid2.transpose(3, 0, 1, 2).reshape(P, n_tiles * 2 * CH_T)
    )
    return idx16, sid2


def assemble_out(res_core: dict, n_tiles: int, chunk: int = CHUNK) -> np.ndarray:
    """Per-chunk transposed outputs -> [n_loc_pad, 128] row-major."""
    parts = []
    for c, cti in enumerate(_chunks(n_tiles, chunk)):
        o = np.asarray(res_core[f"out{c}"], dtype=np.float32)  # [128 fo, cti*128]
        parts.append(
            o.reshape(P, cti, P).transpose(1, 2, 0).reshape(cti * P, XN_OUT)
        )
    return np.concatenate(parts, axis=0)


def balance_tiles(nh_pad: np.ndarray, max_swaps: int = 20000) -> np.ndarray:
    """Permute a core's (padded) nodes across tiles so that every
    (tile, class) edge bucket holds at most SEG edges. Greedy repair:
    swap a high-count node out of the worst violating bucket into the
    emptiest tile for that class. Returns the node permutation
    (perm[i] = padded-node id placed at tile-slot i)."""
    n_pad, _ = nh_pad.shape
    n_tiles = n_pad // P
    cnt = np.stack([(nh_pad % NCLS == j).sum(1) for j in range(NCLS)], 1)
    tiles = np.arange(n_pad).reshape(n_tiles, P)
    bucket = cnt[tiles].sum(1)  # [n_tiles, NCLS]
    for _ in range(max_swaps):
        t, j = np.unravel_index(np.argmax(bucket), bucket.shape)
        if bucket[t, j] <= SEG:
            break
        loc1 = np.argmax(cnt[tiles[t], j])
        i1 = tiles[t, loc1]
        done = False
        for t2 in np.argsort(bucket[:, j])[:32]:
            if t2 == t:
                continue
            loc2 = np.argmin(cnt[tiles[t2], j])
            i2 = tiles[t2, loc2]
            d = cnt[i1] - cnt[i2]
            if d[j] <= 0:
                continue
            if np.all(bucket[t2] + d <= SEG):
                tiles[t, loc1], tiles[t2, loc2] = i2, i1
                bucket[t] -= d
                bucket[t2] += d
                done = True
                break
        assert done, f"balance_tiles stuck: bucket[{t},{j}]={bucket[t, j]}"
    else:
        raise AssertionError("balance_tiles did not converge")
    return tiles.reshape(-1)


def make_in_maps(x, e, ij, Wc, Wn, We, n_cores=N_CORES, ct: int = CT):
    """Host-side shard/prep: per-core input dicts for the SPMD program.

    Returns (in_maps, n_loc, n_loc_pad, perms) where perms[c] is the
    per-core padded-node permutation applied before tiling (so bucket
    caps hold); undo with out_natural[perm] = out_device."""
    import ml_dtypes

    bf = ml_dtypes.bfloat16
    n = e.shape[0]
    n_loc = n // n_cores
    n_loc_pad = ((n_loc + P - 1) // P) * P
    n_tiles = n_loc_pad // P

    x_bf = np.ascontiguousarray(x, dtype=np.float32).astype(bf)
    nh = np.ascontiguousarray(ij[:, :, 0]).astype(np.int32)
    wcT = np.ascontiguousarray(Wc.T, dtype=np.float32).astype(bf)
    wnT = (np.ascontiguousarray(Wn.T, dtype=np.float32) / np.float32(K)).astype(bf)
    weTk = np.ascontiguousarray(We.T, dtype=np.float32) / np.float32(K)
    we2 = np.vstack([weTk, weTk]).astype(bf)  # [128, 128]
    iota = np.broadcast_to(
        np.arange(P, dtype=np.float32), (P, P)
    ).astype(bf).copy()

    in_maps = []
    perms = []
    for c in range(n_cores):
        sl = slice(c * n_loc, (c + 1) * n_loc)
        # padded natural-order per-core arrays; pad rows cycle classes 0..3
        # so they are perfect filler for the balancer
        idx_c = np.tile(np.arange(K, dtype=np.int32) % NCLS, (n_loc_pad, 1))
        idx_c[:n_loc] = nh[sl]
        perm = balance_tiles(idx_c)
        perms.append(perm)
        idx_c = idx_c[perm]

        x_pad = np.zeros((n_loc_pad, XN_IN), bf)
        x_pad[:n_loc] = x_bf[sl]
        x_selfT = np.ascontiguousarray(x_pad[perm].T)
        # e.T blocks: e_loc[p, t*EB*P + blk*P + n] = e[t*P+n, k, f], kf=blk*P+p
        e_kf = np.zeros((n_loc_pad, K * XE_IN), np.float32)
        e_kf[:n_loc] = np.asarray(e[sl], np.float32).reshape(n_loc, K * XE_IN)
        e_kf = e_kf[perm].T
        e_c = np.ascontiguousarray(
            e_kf.reshape(EB, P, n_tiles, P).transpose(1, 2, 0, 3).reshape(
                P, n_tiles * EB * P
            )
        ).astype(ml_dtypes.float8_e4m3)
        idx16, sid2 = prep_gather(idx_c, ct)
        in_maps.append(
            {
                "x_full": x_bf,
                "x_selfT": x_selfT,
                "e_loc": e_c,
                "idx_loc": idx16,
                "sid_loc": sid2.astype(bf),
                "iota_in": iota,
                "wcT": wcT,
                "wnT": wnT,
                "we2": we2,
            }
        )
    return in_maps, n_loc, n_loc_pad, perms


def unpermute(out_dev: np.ndarray, perm: np.ndarray, n_loc: int) -> np.ndarray:
    """Undo a core's balance_tiles permutation on its assembled output."""
    nat = np.empty_like(out_dev)
    nat[perm] = out_dev
    return nat[:n_loc]


def kernel(x, e, ij, Wc, Wn, We):
    x = np.asarray(x)
    e = np.asarray(e)
    ij = np.asarray(ij)
    in_maps, n_loc, n_loc_pad, perms = make_in_maps(x, e, ij, Wc, Wn, We)
    nc = _get_program(n_loc_pad, x.shape[0])
    res = run_bass_kernel_spmd(nc, in_maps, list(range(N_CORES)))
    n_tiles = n_loc_pad // P
    out = np.concatenate(
        [
            unpermute(assemble_out(r, n_tiles), perms[c], n_loc)
            for c, r in enumerate(res.results)
        ],
        axis=0,
    )
    return out.astype(np.float32)


# revision 17
# speedup vs baseline: 1.1519x; 1.1519x over previous
"""Trainium2 Bass kernel: GNN conv block (nn_Conv_block_49331994362308).

Computes, for N=100000 nodes with K=16 neighbors each:
    nh  = ij[:, :, 0]                      # [N, K] neighbor ids
    xnj = mean(x[nh], axis=1)              # neighbor-feature mean  [N, 128]
    xej = mean(e, axis=1)                  # edge-feature mean      [N, 64]
    out = relu(x @ Wc.T + xnj @ Wn.T + xej @ We.T)

Distribution: data-parallel over nodes across 8 NeuronCores (12500 nodes
per core, padded to 12544 = 98*128). x is replicated to every core (in
bf16) so the random neighbor gather x[nh] is a core-local indirect DMA
from HBM.

v2 pipeline (all DMA-visible tensors bf16 except the f32 output; the
kernel is chip-HBM-bandwidth-bound, so bytes are the metric):
  - Neighbor rows arrive via InstDMAGatherAnt on bf16 x directly (no
    f32->bf16 cast stage). dma_gather indices are int16, so x is viewed
    as [N/4, 4, 128] super-rows; the host buckets each tile's 2048 edges
    by nh%4 into fixed 640-slot segments, and one gather instruction per
    (7-tile chunk, class) covers 4480 slots (SWDGE cost is
    994ns + 0.34ns/descriptor, so big gathers amortize the fixed part).
  - Mean-pool of the scrambled gathered rows is 20 accumulating PE
    matmuls per tile against a one-hot matrix P[slot, node] which is
    generated ON DEVICE by a single DVE is_equal over broadcast access
    patterns (iota[n] == sid[slot]); sid is a tiny [slot] -> node id
    stream from the host, stored doubled so every operand keeps a packed
    2-byte innermost axis (DVE 2x mode). Pool output is xnjT [f, n] in
    PSUM; ACT copies it to SBUF bf16.
  - The edge-feature mean never materializes: out.T accumulates
    8 matmuls of We-replicated [kf, fo] against e.T blocks [kf, n]
    (sum over k and f in one PSUM chain), plus Wc@x.T and Wn@xnjT.
    1/K is pre-folded into Wn/We on the host. Output is produced
    transposed [f_out, node]; the host untransposes.
  - DVE applies ReLU into a staging buffer flushed once per 14-tile
    chunk to a once-written per-chunk DRAM tensor (no WAW chains).

Walrus's TRN2 queue-DMA codegen only supports ONE sync-wait command per
DMA (and one per PE LDWEIGHTS), so the structure keeps every DMA at a
single dependency front: indices/sids are preloaded once into SBUF, the
8 SWDGE bookkeeping lanes are warmed with dummy transfers that absorb
the preload front, and warm matmuls give PE a single-wait view of the
weight constants.
"""

from contextlib import ExitStack

import numpy as np

import concourse.bass as bass
import concourse.mybir as mybir
import concourse.tile as tile
from concourse.bass_utils import run_bass_kernel_spmd
from concourse.masks import make_identity
from concourse import library_config

P = 128
K = 16
XN_IN = 128
XE_IN = 64
XN_OUT = 128
N_CORES = 8
N_FULL = 100000
N_LOC = N_FULL // N_CORES          # 12500
N_LOC_PAD = ((N_LOC + P - 1) // P) * P  # 12544
CHUNK = 14                          # tiles per output chunk (98 = 7*14)
CT = 7                              # tiles per gather chunk (98 = 14*7)

F32 = mybir.dt.float32
BF16 = mybir.dt.bfloat16
F8 = mybir.dt.float8e4   # e features; exactness not needed at 2e-2 tolerance
I32 = mybir.dt.int32
I16 = mybir.dt.int16

NCLS = 4           # x rows per int16 "super-row" (mod classes)
# Padded gather slots per (tile, class). 544 = mean 512 + headroom; the
# host rebalances nodes across tiles so no bucket exceeds it. 544 % 128
# = 32, so a tile's class segment always spans exactly NBLK=5 physical
# 128-slot blocks (with halves shared between adjacent tiles; the
# one-hot zeroes foreign slots), keeping PE/DVE work identical to an
# aligned grid while cutting gather descriptors and bytes by 15%.
SEG = 544
import math as _math
_G = _math.gcd(SEG, P)
# worst-case 128-slot blocks a tile's class segment touches
NBLK = max((r + SEG + P - 1) // P for r in range(0, P, _G))
CH_T = NBLK * NCLS  # pool blocks per tile = 20
EB = K * XE_IN // P  # e.T blocks per tile = 8


def _stream_len(ct: int) -> int:
    """Gather slots per (chunk, class), rounded to full 128-slot blocks so
    the last output block is fully written (trailing pads gather row 0)."""
    return -(-ct * SEG // P) * P


def _chunks(n_tiles: int, chunk: int = CHUNK) -> list[int]:
    out = []
    t = 0
    while t < n_tiles:
        out.append(min(chunk, n_tiles - t))
        t += chunk
    return out


def build_program(
    n_loc_pad: int, n_src: int, ct: int = CT, chunk: int = CHUNK, reps: int = 1
) -> bass.Bass:
    """Build the SPMD per-core Bass program (same program on every core).

    reps > 1 duplicates every tile body (identical I/O) — a timing probe:
    wall(reps=2) - wall(reps=1) isolates one pass's true execution time.
    """
    assert n_loc_pad % P == 0 and n_src % NCLS == 0
    n_tiles = n_loc_pad // P
    assert n_tiles % ct == 0
    chunks = _chunks(n_tiles, chunk)
    n_gch = n_tiles // ct
    sl = _stream_len(ct)
    seg16 = sl // 16  # idx16 columns per (gather chunk, class)

    # detect_race_conditions=False: the post-schedule wait-legalizer's nop
    # carriers share scratch tiles and trip the sim race detector's
    # bookkeeping (same-engine program order makes them safe).
    nc = bass.Bass("TRN2", debug=False, detect_race_conditions=False)

    x_full = nc.dram_tensor("x_full", [n_src, XN_IN], BF16, kind="ExternalInput").ap()
    x_selfT = nc.dram_tensor("x_selfT", [P, n_loc_pad], BF16, kind="ExternalInput").ap()
    # e.T blocks: e_loc[p, t*EB*P + blk*P + n] = e[t*P+n, k, f], kf = blk*P+p
    e_loc = nc.dram_tensor("e_loc", [P, n_tiles * EB * P], F8, kind="ExternalInput").ap()
    # int16 super-row ids (nh//4), wrapped [16, L/16] + replicated to 128
    # partitions, concatenated over (gather chunk, class)
    idx_loc = nc.dram_tensor(
        "idx_loc", [P, n_gch * NCLS * seg16], I16, kind="ExternalInput"
    ).ap()
    # per-slot node ids (-1 for dead slots), doubled along the free axis so
    # the one-hot is_equal keeps a packed 2-byte innermost pair on every
    # operand (DVE 2x mode): sid[p, t*2*CH_T + 2*b + r] = node of slot
    # (b*128+p) of tile t, r in {0,1}
    sid_loc = nc.dram_tensor(
        "sid_loc", [P, n_tiles * 2 * CH_T], BF16, kind="ExternalInput"
    ).ap()
    iota_in = nc.dram_tensor("iota_in", [P, P], BF16, kind="ExternalInput").ap()
    wcT = nc.dram_tensor("wcT", [XN_IN, XN_OUT], BF16, kind="ExternalInput").ap()
    wnT = nc.dram_tensor("wnT", [XN_IN, XN_OUT], BF16, kind="ExternalInput").ap()
    # We.T/K duplicated on partitions: we2[p, fo] = We[fo, p % 64] / K
    we2 = nc.dram_tensor("we2", [P, XN_OUT], BF16, kind="ExternalInput").ap()
    # per-chunk outputs, transposed: out_c[p, i*128+n] = out[(t0+i)*128+n, p]
    outs = [
        nc.dram_tensor(f"out{c}", [P, cti * P], BF16, kind="ExternalOutput").ap()
        for c, cti in enumerate(chunks)
    ]

    nop_sem = nc.alloc_semaphore("waitnop")

    with tile.TileContext(nc) as tc, ExitStack() as ctx:
        nc.gpsimd.sem_clear(range(nop_sem.num, nop_sem.num + 1))
        nc.gpsimd.load_library(library_config.mlp)
        consts = ctx.enter_context(tc.tile_pool(name="consts", bufs=1))
        ident_bf = consts.tile([P, P], BF16, tag="ident_bf")
        make_identity(nc, ident_bf[:])
        wcT_sb = consts.tile([XN_IN, XN_OUT], BF16, tag="wc")
        wnT_sb = consts.tile([XN_IN, XN_OUT], BF16, tag="wn")
        we2_sb = consts.tile([P, XN_OUT], BF16, tag="we2")
        iota_sb = consts.tile([P, P], BF16, tag="iota")
        nc.sync.dma_start(wcT_sb[:], wcT[:, :])
        nc.sync.dma_start(wnT_sb[:], wnT[:, :])
        nc.sync.dma_start(we2_sb[:], we2[:, :])
        nc.sync.dma_start(iota_sb[:], iota_in[:, :])
        idx_all = consts.tile([P, n_gch * NCLS * seg16], I16, tag="idx_all")
        nc.sync.dma_start(idx_all[:], idx_loc[:, :])
        sid_all = consts.tile([P, n_tiles * 2 * CH_T], BF16, tag="sid_all")
        nc.sync.dma_start(sid_all[:], sid_loc[:, :])
        # x viewed as [n_src/4, 4, 128]: class j gathers row 4*i16+j via
        # elem_step=512 elements (1024B stride) and a j*128-element offset
        x4 = x_full.rearrange("(r c) f -> r c f", c=NCLS)

        # Warm the 8 SWDGE bookkeeping lanes: each dummy absorbs the
        # idx-preload front so later gathers carry only their PE front.
        scratch = ctx.enter_context(tc.tile_pool(name="scratch", bufs=1))
        for q in range(8):
            sc = scratch.tile([1, K], I16, tag=f"sc{q}")
            nc.gpsimd.dma_start(sc[:], idx_all[:1, :K])
        # Tiny template instructions for _legalize_waits nop carriers
        # (one per DMA queue and per compute engine).
        nop_hw = scratch.tile([1, K], I16, tag="noptpl_hw")
        nc.sync.dma_start(nop_hw[:], idx_loc[:1, :K])
        nop_sw = scratch.tile([1, K], I16, tag="noptpl_sw")
        nc.gpsimd.dma_start(nop_sw[:], idx_loc[:1, :K])
        nop_sw_act = scratch.tile([1, K], I16, tag="noptpl_sw_act")
        nc.scalar.dma_start(nop_sw_act[:], idx_loc[:1, :K])
        nop_dve = scratch.tile([P, K], BF16, tag="noptpl_dve")
        nc.vector.tensor_copy(nop_dve[:], ident_bf[:, :K])
        nop_act = scratch.tile([P, K], BF16, tag="noptpl_act")
        nc.scalar.copy(nop_act[:], ident_bf[:, :K])
        nop_pool = scratch.tile([P, K], F32, tag="noptpl_pool")
        nc.gpsimd.memset(nop_pool[:], 0.0)

        g_pool = ctx.enter_context(tc.tile_pool(name="gatherp", bufs=2))
        pp_pool = ctx.enter_context(tc.tile_pool(name="poolmat", bufs=3))
        e_pool = ctx.enter_context(tc.tile_pool(name="edgep", bufs=3))
        xs_pool = ctx.enter_context(tc.tile_pool(name="xselfp", bufs=4))
        st_pool = ctx.enter_context(tc.tile_pool(name="stagep", bufs=3))
        out_pool = ctx.enter_context(tc.tile_pool(name="outp", bufs=2))
        psum_pool = ctx.enter_context(tc.tile_pool(name="psump", bufs=2, space="PSUM"))
        psum1_pool = ctx.enter_context(tc.tile_pool(name="psum1p", bufs=1, space="PSUM"))

        # Warm up PE's view of the constants so steady-state matmuls carry at
        # most one sync wait (PE LDWEIGHTS supports a single wait command).
        ps_warm = psum1_pool.tile([P, P], F32, tag="warm")
        nc.tensor.matmul(ps_warm[:], ident_bf[:], ident_bf[:], start=True, stop=False)
        nc.tensor.matmul(ps_warm[:], wcT_sb[:], ident_bf[:], start=False, stop=False)
        nc.tensor.matmul(ps_warm[:], wnT_sb[:], ident_bf[:], start=False, stop=False)
        nc.tensor.matmul(ps_warm[:], we2_sb[:], ident_bf[:], start=False, stop=True)

        t = 0
        gb = [None] * NCLS
        nidx_reg = nc.gpsimd.to_reg(sl)  # shared across all gathers
        for c, cti in enumerate(chunks):
            o_stage = out_pool.tile([P, cti * P], BF16, tag="ostage")
            for i0 in range(cti * reps):
                rep, i = divmod(i0, cti) if reps > 1 else (0, i0)
                t = (c * chunk) + i
                g, ti = divmod(t, ct)

                if ti == 0:
                    # one gather per class covering this ct-tile chunk;
                    # slot s lands at partition s%128, free block s//128, so
                    # each tile's 5 blocks per class stay 128-aligned.
                    for j in range(NCLS):
                        off = (g * NCLS + j) * seg16
                        gout = g_pool.tile([P, sl], BF16, tag=f"go{j}")
                        nc.gpsimd.dma_gather(
                            out_ap=gout[:].rearrange("p (b f) -> p b f", f=XN_IN),
                            in_ap=x4[:, j, :],
                            idxs_ap=idx_all[:, off:off + seg16],
                            num_idxs=sl,
                            num_idxs_reg=nidx_reg,
                            elem_size=XN_IN,
                            elem_step=NCLS * XN_IN,
                            single_packet=False,
                        )
                        gb[j] = gout

                # one-hot P[slot, node] = (iota[node] == sid[slot]), one DVE
                # op per tile over broadcast APs with packed innermost pairs
                p_t = pp_pool.tile([P, CH_T * P], BF16, tag="pmat")
                o_ap = p_t[:].rearrange("p (b nh nl) -> p b nh nl", b=CH_T, nl=2)
                i_ap = (
                    iota_sb[:]
                    .rearrange("p (o nh nl) -> p o nh nl", o=1, nl=2)
                    .to_broadcast([P, CH_T, P // 2, 2])
                )
                s_ap = (
                    sid_all[:, t * 2 * CH_T:(t + 1) * 2 * CH_T]
                    .rearrange("p (b nl) -> p b nl", nl=2)
                    .unsqueeze(2)
                    .to_broadcast([P, CH_T, P // 2, 2])
                )
                nc.vector.tensor_tensor(
                    o_ap, i_ap, s_ap, op=mybir.AluOpType.is_equal
                )

                # xnjT[f, n] = sum_slot g[slot, f] * P[slot, n]
                xnjT_ps = psum_pool.tile([P, P], F32, tag="ps_xnj")
                for b in range(CH_T):
                    j, bl = divmod(b, NBLK)
                    blk = (SEG * ti) // P + bl
                    nc.tensor.matmul(
                        xnjT_ps[:],
                        gb[j][:, blk * XN_IN:(blk + 1) * XN_IN],
                        p_t[:, b * P:(b + 1) * P],
                        start=(b == 0),
                        stop=(b == CH_T - 1),
                    )
                xnjT_bf = st_pool.tile([P, P], BF16, tag="xnj_bf")
                nc.scalar.copy(xnjT_bf[:], xnjT_ps[:])

                x_sb = xs_pool.tile([P, P], BF16, tag="xT")
                nc.sync.dma_start(x_sb[:], x_selfT[:, t * P:(t + 1) * P])
                e_sb = e_pool.tile([P, EB * P], F8, tag="e")
                nc.scalar.dma_start(
                    e_sb[:], e_loc[:, t * EB * P:(t + 1) * EB * P]
                )

                # outT[fo, n] = sum_f wcT[f,fo] xT[f,n] + wnT[f,fo] xnjT[f,n]
                #             + sum_kf we2[kf,fo] eT[kf,n]
                outT_ps = psum_pool.tile([P, P], F32, tag="ps_out")
                for blk in range(EB):
                    nc.tensor.matmul(
                        outT_ps[:],
                        we2_sb[:],
                        e_sb[:, blk * P:(blk + 1) * P],
                        start=(blk == 0),
                        stop=False,
                    )
                nc.tensor.matmul(outT_ps[:], wcT_sb[:], x_sb[:], start=False, stop=False)
                nc.tensor.matmul(outT_ps[:], wnT_sb[:], xnjT_bf[:], start=False, stop=True)

                # ReLU on DVE into the chunk staging buffer
                nc.vector.tensor_scalar_max(
                    o_stage[:, i * P:(i + 1) * P], outT_ps[:], 0.0
                )

            nc.sync.dma_start(outs[c][:, :], o_stage[:])

    from concourse.library_overlay import lower_extended_insts

    lower_extended_insts(nc)
    _legalize_waits(nc, nop_sem)
    return nc


def _legalize_waits(nc: bass.Bass, nop_sem) -> None:
    """Split multi-wait queue-DMAs / matmuls for walrus's 1-wait codegen limit.

    The TRN2 walrus codegen allows a single sync-wait command per queue-DMA
    entry and per PE matmul (S3_LW struct). Tile emits minimal waits but can
    still produce 2+ (e.g. a slot's previous-writer DMA completion plus its
    last-reader engine release — Tile's clocks are not transitive). Queue
    entries execute in FIFO order, so extra waits are moved onto tiny no-op
    carrier DMAs inserted immediately before the offender on the same queue.
    For matmuls the carrier is a 1-column bf16 LDWEIGHTS (any clobbered
    weights are reloaded by each matmul's own weight load; insertion happens
    before a directly-preceding LDWEIGHTS so split LDW+MM pairs stay intact).
    """
    import copy

    dma_tpl: dict = {}
    eng_tpl: dict = {}
    evsem_tpl: dict = {}
    ldw_tpl = None
    for f in nc.m.functions:
        for blk in f.blocks:
            for inst in blk.instructions:
                tn = type(inst).__name__
                dst = (
                    str(getattr(inst.outs[0], "memref", "")) if inst.outs else ""
                )
                if tn == "InstDMACopy":
                    if dst.startswith("nop_hw"):
                        dma_tpl["qSPDynamicHW"] = inst
                    elif dst.startswith("nop_sw"):
                        dma_tpl[inst.queue] = inst
                elif tn == "InstLdweights" and ldw_tpl is None:
                    ldw_tpl = inst
                elif tn == "InstEventSemaphore":
                    evsem_tpl[inst.engine] = inst
                elif dst.startswith("nop_dve") or dst.startswith("nop_act") or dst.startswith("nop_pool"):
                    eng_tpl[inst.engine] = inst

    counter = [0]

    def make_nop(tpl, wait):
        counter[0] += 1
        nop = copy.deepcopy(tpl)
        nop.name = f"I-{nc.next_id()}"
        # DMA carriers must update a semaphore (BIR invariant); use a
        # dedicated one nobody waits on. Other engines' carriers stay
        # update-free (walrus rejects a waitnop update on e.g. TensorCopy
        # with a no_semaphore_value_conflict ISA check).
        upd = []
        if type(tpl).__name__ == "InstDMACopy":
            upd = [
                mybir.SyncUpdate(
                    sync_type="semaphore",
                    id=nop_sem.num,
                    ant_name=nop_sem.name,
                    update_mode="sem-add-imm",
                    update_value=16,
                )
            ]
        nop.sync_info = mybir.SyncInfo(on_wait=[wait], on_update=upd)
        nc.inst_map[nop.name] = nop
        return nop

    for f in nc.m.functions:
        for blk in f.blocks:
            out: list = []
            changed = False
            insts = list(blk.instructions)
            for pos, inst in enumerate(insts):
                tn = type(inst).__name__
                si = inst.sync_info
                waits = list(si.on_wait) if si else []
                nops = None
                if len(waits) > 1:
                    if tn == "InstDMACopy":
                        tpl = dma_tpl.get(inst.queue)
                        assert tpl is not None, f"no nop template for {inst.queue}"
                        nops = [make_nop(tpl, w) for w in waits[:-1]]
                    elif tn in ("InstMatmult", "InstLdweights"):
                        assert ldw_tpl is not None, "no ldweights template"
                        nops = [make_nop(ldw_tpl, w) for w in waits[:-1]]
                        # keep split LDW+MM pairs adjacent
                        if out and type(out[-1]).__name__ == "InstLdweights":
                            own_ldw = out.pop()
                            nops.append(own_ldw)
                    elif tn == "InstDrain":
                        # a drain is its own carrier: extra single-wait drains
                        # on the same engine are harmless
                        nops = [make_nop(inst, w) for w in waits[:-1]]
                    elif inst.engine in eng_tpl and tn not in (
                        "InstDrain",
                        "InstEventSemaphore",
                        "InstSemaphoreOp",
                    ):
                        nops = [make_nop(eng_tpl[inst.engine], w) for w in waits[:-1]]
                if nops:
                    out.extend(nops)
                    inst.sync_info = mybir.SyncInfo(
                        on_wait=waits[-1:], on_update=list(si.on_update)
                    )
                    changed = True
                out.append(inst)
            if changed:
                try:
                    blk.instructions[:] = out
                except TypeError:
                    blk.instructions.clear()
                    blk.instructions.extend(out)


_PROGRAM_CACHE: dict = {}


def _get_program(n_loc_pad: int, n_src: int, ct: int = CT, chunk: int = CHUNK) -> bass.Bass:
    key = (n_loc_pad, n_src, ct, chunk)
    if key not in _PROGRAM_CACHE:
        _PROGRAM_CACHE[key] = build_program(n_loc_pad, n_src, ct, chunk)
    return _PROGRAM_CACHE[key]


def prep_gather(nh_pad: np.ndarray, ct: int = CT):
    """Bucket edges by nh%4 per tile into fixed SEG-slot segments of the
    per-(chunk, class) gather stream; emit the int16 super-row stream
    (wrapped [16, L/16], replicated to 128 partitions) and the doubled
    per-slot node-id stream for on-device one-hot generation. Tile
    segments need not be 128-aligned: sid maps each of the NBLK touched
    physical blocks, with -1 for slots owned by the adjacent tile.

    Returns (idx16 [128, n_gch*NCLS*seg16], sid2 [128, n_tiles*2*CH_T] f32).
    """
    n_pad = nh_pad.shape[0]
    n_tiles = n_pad // P
    assert n_tiles % ct == 0
    n_gch = n_tiles // ct
    sl = _stream_len(ct)
    seg16 = sl // 16

    idx16 = np.zeros((n_gch * NCLS, sl), np.int16)
    sid = np.full((n_tiles, CH_T * P), -1.0, np.float32)  # [tile, slot] -> node
    for t in range(n_tiles):
        nh_t = nh_pad[t * P:(t + 1) * P]          # [128 nodes, K]
        nodes = np.repeat(np.arange(P), K)         # edge -> node
        vals = nh_t.reshape(-1)                    # edge -> neighbor id
        cls = vals % NCLS
        g, ti = divmod(t, ct)
        base = ti * SEG           # segment start within the chunk stream
        sb = (SEG * ti) // P      # first physical block the tile touches
        for j in range(NCLS):
            sel = np.nonzero(cls == j)[0]
            l = len(sel)
            assert l <= SEG, f"class overflow {l} > {SEG}"
            idx16[g * NCLS + j, base:base + l] = (vals[sel] // NCLS).astype(
                np.int16
            )
            o = np.arange(l)
            pb = (base + o) // P          # physical block of each slot
            pp = (base + o) % P           # partition of each slot
            sid[t, (j * NBLK + pb - sb) * P + pp] = nodes[sel]
    # wrap idx16: entry i -> [i%16, i//16]; replicate 16-row block to 128
    idx16 = idx16.reshape(n_gch * NCLS, sl // 16, 16).transpose(0, 2, 1)
    idx16 = np.tile(idx16, (1, 8, 1)).reshape(n_gch, NCLS, P, seg16)
    idx16 = np.ascontiguousarray(
        idx16.transpose(2, 0, 1, 3).reshape(P, n_gch * NCLS * seg16)
    )
    # sid2[p, t*2*CH_T + 2*b + r] = sid[t, b*128 + p]
    sid_b = sid.reshape(n_tiles, CH_T, P)          # [t, b, p]
    sid2 = np.repeat(sid_b, 2, axis=1).reshape(n_tiles, CH_T, 2, P)
    sid2 = np.ascontiguousarray(
        sid2.transpose(3, 0, 1, 2).reshape(P, n_tiles * 2 * CH_T)
    )
    return idx16, sid2


def assemble_out(res_core: dict, n_tiles: int, chunk: int = CHUNK) -> np.ndarray:
    """Per-chunk transposed outputs -> [n_loc_pad, 128] row-major."""
    parts = []
    for c, cti in enumerate(_chunks(n_tiles, chunk)):
        o = np.asarray(res_core[f"out{c}"], dtype=np.float32)  # [128 fo, cti*128]
        parts.append(
            o.reshape(P, cti, P).transpose(1, 2, 0).reshape(cti * P, XN_OUT)
        )
    return np.concatenate(parts, axis=0)


def balance_tiles(nh_pad: np.ndarray, max_swaps: int = 20000) -> np.ndarray:
    """Permute a core's (padded) nodes across tiles so that every
    (tile, class) edge bucket holds at most SEG edges. Greedy repair:
    swap a high-count node out of the worst violating bucket into the
    emptiest tile for that class. Returns the node permutation
    (perm[i] = padded-node id placed at tile-slot i)."""
    n_pad, _ = nh_pad.shape
    n_tiles = n_pad // P
    cnt = np.stack([(nh_pad % NCLS == j).sum(1) for j in range(NCLS)], 1)
    tiles = np.arange(n_pad).reshape(n_tiles, P)
    bucket = cnt[tiles].sum(1)  # [n_tiles, NCLS]
    for _ in range(max_swaps):
        t, j = np.unravel_index(np.argmax(bucket), bucket.shape)
        if bucket[t, j] <= SEG:
            break
        loc1 = np.argmax(cnt[tiles[t], j])
        i1 = tiles[t, loc1]
        done = False
        for t2 in np.argsort(bucket[:, j])[:32]:
            if t2 == t:
                continue
            loc2 = np.argmin(cnt[tiles[t2], j])
            i2 = tiles[t2, loc2]
            d = cnt[i1] - cnt[i2]
            if d[j] <= 0:
                continue
            if np.all(bucket[t2] + d <= SEG):
                tiles[t, loc1], tiles[t2, loc2] = i2, i1
                bucket[t] -= d
                bucket[t2] += d
                done = True
                break
        assert done, f"balance_tiles stuck: bucket[{t},{j}]={bucket[t, j]}"
    else:
        raise AssertionError("balance_tiles did not converge")
    return tiles.reshape(-1)


def make_in_maps(x, e, ij, Wc, Wn, We, n_cores=N_CORES, ct: int = CT):
    """Host-side shard/prep: per-core input dicts for the SPMD program.

    Returns (in_maps, n_loc, n_loc_pad, perms) where perms[c] is the
    per-core padded-node permutation applied before tiling (so bucket
    caps hold); undo with out_natural[perm] = out_device."""
    import ml_dtypes

    bf = ml_dtypes.bfloat16
    n = e.shape[0]
    n_loc = n // n_cores
    n_loc_pad = ((n_loc + P - 1) // P) * P
    n_tiles = n_loc_pad // P

    x_bf = np.ascontiguousarray(x, dtype=np.float32).astype(bf)
    nh = np.ascontiguousarray(ij[:, :, 0]).astype(np.int32)
    wcT = np.ascontiguousarray(Wc.T, dtype=np.float32).astype(bf)
    wnT = (np.ascontiguousarray(Wn.T, dtype=np.float32) / np.float32(K)).astype(bf)
    weTk = np.ascontiguousarray(We.T, dtype=np.float32) / np.float32(K)
    we2 = np.vstack([weTk, weTk]).astype(bf)  # [128, 128]
    iota = np.broadcast_to(
        np.arange(P, dtype=np.float32), (P, P)
    ).astype(bf).copy()

    in_maps = []
    perms = []
    for c in range(n_cores):
        sl = slice(c * n_loc, (c + 1) * n_loc)
        # padded natural-order per-core arrays; pad rows cycle classes 0..3
        # so they are perfect filler for the balancer
        idx_c = np.tile(np.arange(K, dtype=np.int32) % NCLS, (n_loc_pad, 1))
        idx_c[:n_loc] = nh[sl]
        perm = balance_tiles(idx_c)
        perms.append(perm)
        idx_c = idx_c[perm]

        x_pad = np.zeros((n_loc_pad, XN_IN), bf)
        x_pad[:n_loc] = x_bf[sl]
        x_selfT = np.ascontiguousarray(x_pad[perm].T)
        # e.T blocks: e_loc[p, t*EB*P + blk*P + n] = e[t*P+n, k, f], kf=blk*P+p
        e_kf = np.zeros((n_loc_pad, K * XE_IN), np.float32)
        e_kf[:n_loc] = np.asarray(e[sl], np.float32).reshape(n_loc, K * XE_IN)
        e_kf = e_kf[perm].T
        e_c = np.ascontiguousarray(
            e_kf.reshape(EB, P, n_tiles, P).transpose(1, 2, 0, 3).reshape(
                P, n_tiles * EB * P
            )
        ).astype(ml_dtypes.float8_e4m3)
        idx16, sid2 = prep_gather(idx_c, ct)
        in_maps.append(
            {
                "x_full": x_bf,
                "x_selfT": x_selfT,
                "e_loc": e_c,
                "idx_loc": idx16,
                "sid_loc": sid2.astype(bf),
                "iota_in": iota,
                "wcT": wcT,
                "wnT": wnT,
                "we2": we2,
            }
        )
    return in_maps, n_loc, n_loc_pad, perms


def unpermute(out_dev: np.ndarray, perm: np.ndarray, n_loc: int) -> np.ndarray:
    """Undo a core's balance_tiles permutation on its assembled output."""
    nat = np.empty_like(out_dev)
    nat[perm] = out_dev
    return nat[:n_loc]


def kernel(x, e, ij, Wc, Wn, We):
    x = np.asarray(x)
    e = np.asarray(e)
    ij = np.asarray(ij)
    in_maps, n_loc, n_loc_pad, perms = make_in_maps(x, e, ij, Wc, Wn, We)
    nc = _get_program(n_loc_pad, x.shape[0])
    res = run_bass_kernel_spmd(nc, in_maps, list(range(N_CORES)))
    n_tiles = n_loc_pad // P
    out = np.concatenate(
        [
            unpermute(assemble_out(r, n_tiles), perms[c], n_loc)
            for c, r in enumerate(res.results)
        ],
        axis=0,
    )
    return out.astype(np.float32)
